# revision 1
# baseline (speedup 1.0000x reference)
"""Trainium2 Bass kernel for nn_ContentAttention.

reference:
    bias = (aspect @ aspect_w + sentence @ sent_w)[:, None, :]        # [B,1,D]
    h    = tanh(context @ context_w + bias)                           # [B,T,D]
    g    = h @ attend_w[:, 0]                                         # [B,T]
    a    = exp(g) * mask;  a = a / (sum(a) + 1e-7)
    out  = einsum('btd,bt->bd', context, a) + sentence                # [B,D]

Strategy: data-parallel over batch across 8 cores (8 batches/core), weights
replicated.  The normalization is deferred (divide the unnormalized weighted
sum by the unnormalized denominator at the end) so context is read exactly
once from HBM.  Matmuls run in float32r (FP22 single-pass).  Per 512-token
strip: PE-transpose ctx -> mm1 (z^T = W^T ctx^T) -> tanh(+bias) on ACT ->
mm2 (g = v^T h^T) col-packed by strip into one PSUM bank per batch.  Batch
tail: exp on ACT, transpose w into per-block columns, mask multiply, then
mm3 (w^T as stationary, natural ctx tiles + ones column as moving operand)
accumulates the weighted sum and the mask-weighted denominator together.
"""

import sys

if "/opt/trn_rl_repo" not in sys.path:
    sys.path.insert(0, "/opt/trn_rl_repo")

import numpy as np

import concourse.bass as bass
import concourse.tile as tile
from concourse import mybir
from concourse import bass_utils
from concourse.masks import make_identity
from concourse.tile import ScopedClock

# ---------------------------------------------------------------------------
# Workaround for this neuronxcc build: InstDrain carries at most ~1 sync wait
# ("Too many sync wait commands" in walrus codegen otherwise).  TileContext's
# tail drain collects one wait per outstanding proc; split them across a
# chain of drains, one wait each.
# ---------------------------------------------------------------------------


def _drain_and_barrier_split(self, tick_clock, wait_clock):
    drain_inst = self.nc.sync.drain()
    wait_clock.add_sem_waits(
        drain_inst.ins, ScopedClock({None: tick_clock.global_clock})
    )
    si = drain_inst.ins.sync_info
    waits = list(si.on_wait) if si is not None and si.on_wait else []
    if len(waits) > 1:
        si.on_wait = [waits[0]]
        for w in waits[1:]:
            extra = self.nc.sync.drain()
            esi = extra.ins.sync_info
            if esi is None:
                extra.ins.sync_info = mybir.SyncInfo(on_wait=[w], on_update=[])
            else:
                esi.on_wait = list(esi.on_wait) + [w]

    self.nc.all_engine_barrier()
    assert self.sems is not None
    popped = self.nc._tile_sem_poison_stack.pop()
    assert popped is self._sem_poison
    self.nc.clear_and_free_semaphores(list(self.sems.allocated().values()))
    self.nc.all_engine_barrier()


tile.TileContext._drain_and_barrier = _drain_and_barrier_split


# This walrus build also rejects multi-wait Matmult (S3_LW struct).  After
# Tile scheduling, hoist excess sync waits from any instruction onto
# injected single-wait drains just before it (same engine stream, so the
# semantics are identical: the engine blocks on every wait either way).
_WAIT_CAPS = {"InstMatmult": 1, "InstLdweights": 1, "InstDrain": 1}
_DEFAULT_WAIT_CAP = 1


def _split_excess_waits(nc):
    uid = 0
    for blk in nc.m.functions[0].blocks:
        new_insts = []
        for inst in blk.instructions:
            si = getattr(inst, "sync_info", None)
            nw = len(si.on_wait) if si is not None and si.on_wait else 0
            cap = _WAIT_CAPS.get(type(inst).__name__, _DEFAULT_WAIT_CAP)
            if nw > cap:
                waits = list(si.on_wait)
                for w in waits[:-cap]:
                    d = mybir.InstDrain(name=f"I-wsplit-{uid}", ins=[], outs=[])
                    uid += 1
                    d.engine = inst.engine
                    d.sync_info = mybir.SyncInfo(on_wait=[w], on_update=[])
                    new_insts.append(d)
                si.on_wait = waits[-cap:]
            new_insts.append(inst)
        blk.instructions[:] = new_insts


# ---------------------------------------------------------------------------

B, T, D = 64, 2048, 256
NCORES = 8
BPC = B // NCORES          # batches per core
NSTRIP = T // 512          # 512-token strips per batch
NRING = 12                 # persistent ctx tiles (3 batches of lookahead)
EPS = 1e-7

F32 = mybir.dt.float32
F32R = mybir.dt.float32r
U8 = mybir.dt.uint8
AF = mybir.ActivationFunctionType


def build_program(reps: int = 1, split_waits: bool = True) -> bass.Bass:
    nc = bass.Bass("TRN2", target_bir_lowering=False, debug=False,
                   num_devices=NCORES)

    ctx_d = nc.dram_tensor("context", [BPC, T, D], F32, kind="ExternalInput").ap()
    asp_d = nc.dram_tensor("aspect", [BPC, D], F32, kind="ExternalInput").ap()
    sen_d = nc.dram_tensor("sentence", [BPC, D], F32, kind="ExternalInput").ap()
    msk_d = nc.dram_tensor("mask", [BPC, T], U8, kind="ExternalInput").ap()
    ctxw_d = nc.dram_tensor("ctxw", [D, D], F32, kind="ExternalInput").ap()
    aspw_d = nc.dram_tensor("aspw", [D, D], F32, kind="ExternalInput").ap()
    senw_d = nc.dram_tensor("senw", [D, D], F32, kind="ExternalInput").ap()
    attw_d = nc.dram_tensor("attw", [D, 1], F32, kind="ExternalInput").ap()
    out_d = nc.dram_tensor("out", [BPC, D], F32, kind="ExternalOutput").ap()

    with tile.TileContext(nc) as tc:
        with (
            tc.tile_pool(name="consts", bufs=1) as consts,
            tc.tile_pool(name="ring", bufs=NRING) as ring_pool,
            tc.tile_pool(name="work", bufs=2) as work,
            tc.tile_pool(name="hwork", bufs=2) as hwork,
            tc.tile_pool(name="p_tr", bufs=2, space="PSUM") as p_tr,
            tc.tile_pool(name="p_z", bufs=2, space="PSUM") as p_z,
            tc.tile_pool(name="p_gw", bufs=2, space="PSUM") as p_gw,
            tc.tile_pool(name="p_att", bufs=2, space="PSUM") as p_att,
        ):
            # ---- constants -------------------------------------------------
            ident = consts.tile([128, 128], F32, name="ident")
            make_identity(nc, ident)
            eps_t = consts.tile([1, 1], F32, name="eps_t")
            nc.vector.memset(eps_t, EPS)

            wq = {}
            for nm, dr in (("ctxw", ctxw_d), ("aspw", aspw_d), ("senw", senw_d)):
                tl = consts.tile([128, 2, 2, 128], F32R, name=f"{nm}_sb")
                nc.sync.dma_start(
                    out=tl,
                    in_=dr.rearrange("(c p) (u e) -> p c u e", p=128, u=2)
                    .bitcast(F32R),
                )
                wq[nm] = tl

            # rows 0:8 aspect, 8:16 sentence, 16 attend_w
            stack_sb = consts.tile([17, 256], F32, name="stack_sb")
            nc.sync.dma_start(out=stack_sb[0:8, :], in_=asp_d)
            nc.sync.dma_start(out=stack_sb[8:16, :], in_=sen_d)
            nc.sync.dma_start(out=stack_sb[16:17, :],
                              in_=attw_d.rearrange("d one -> one d"))

            # sentence rows on partition 0 for the final fixup
            sen_row = consts.tile([1, BPC, 256], F32, name="sen_row")
            nc.sync.dma_start(out=sen_row, in_=sen_d.unsqueeze(0))

            out_sb = consts.tile([1, BPC, 256], F32, name="out_sb")

            # ---- persistent ctx tiles with a ones column -------------------
            ctx_ring = []
            for i in range(NRING):
                t = ring_pool.tile([128, 4, 258], F32, name=f"ctx{i}", tag="ctx")
                nc.vector.memset(t[:, :, 256:258], 1.0)
                ctx_ring.append(t)

            # ---- stackT: transpose aspect/sentence/v -----------------------
            # stackT_sb[:, c, 0:8]=aspect^T, [:, c, 8:16]=sentence^T,
            # [:, c, 16]=attend_w chunk c   (partition = d within chunk c)
            stackT_sb = consts.tile([128, 2, 17], F32R, name="stackT_sb")
            pst = p_tr.tile([128, 256], F32, tag="ptr")
            for c in range(2):
                nc.tensor.matmul(
                    out=pst[:, 17 * c:17 * (c + 1)],
                    lhsT=stack_sb[:, 128 * c:128 * (c + 1)],
                    rhs=ident[0:17, 0:17],
                    is_transpose=True, start=(c == 0), stop=(c == 1),
                )
            nc.vector.tensor_copy(out=stackT_sb, in_=pst[:, 0:34])

            # ---- biasT[e, b] = (aspect @ aspw + sentence @ senw)^T ---------
            pbias = p_tr.tile([128, 256], F32, tag="ptr")
            steps = []
            for c2 in range(2):
                for c in range(2):
                    for wn, off in (("aspw", 0), ("senw", 8)):
                        steps.append((c2, c, wn, off))
            for i, (c2, c, wn, off) in enumerate(steps):
                nc.tensor.matmul(
                    out=pbias[:, 8 * c2:8 * (c2 + 1)],
                    lhsT=wq[wn][:, c, c2, :],
                    rhs=stackT_sb[:, c, off:off + 8],
                    start=(i == 0), stop=(i == len(steps) - 1),
                )
            biasT_sb = consts.tile([128, 16], F32, name="biasT_sb")
            nc.vector.tensor_copy(out=biasT_sb, in_=pbias[:, 0:16])

            # ---- main loop -------------------------------------------------
            for rep in range(reps):
                for b in range(BPC):
                    gidx = rep * BPC + b
                    # -------- pass A: attention logits g --------------------
                    w4 = work.tile([128, 512], F32, tag="w4")
                    for s in range(NSTRIP):
                        ct = ctx_ring[(gidx * NSTRIP + s) % NRING]
                        nc.sync.dma_start(
                            out=ct[:, :, 0:256].bitcast(F32R),
                            in_=ctx_d[b, 512 * s:512 * (s + 1), :]
                                .rearrange("(j p) d -> p j d", p=128)
                                .bitcast(F32R),
                        )
                        # ctxT[p, j, c, ti] = ctx[d=128c+p, t=128j+ti]
                        ctxT = work.tile([128, 4, 2, 128], F32R, tag="ctxT")
                        for j in range(4):
                            ptr = p_tr.tile([128, 256], F32, tag="ptr")
                            for c in range(2):
                                nc.tensor.matmul(
                                    out=ptr[:, 128 * c:128 * (c + 1)],
                                    lhsT=ct[:, j, 128 * c:128 * (c + 1)],
                                    rhs=ident,
                                    is_transpose=True,
                                    start=(c == 0), stop=(c == 1),
                                )
                            nc.vector.tensor_copy(out=ctxT[:, j, :, :], in_=ptr)
                        hT = hwork.tile([128, 2, 512], F32R, tag="hT")
                        for c2 in range(2):
                            z = p_z.tile([128, 512], F32, tag="z")
                            for c in range(2):
                                nc.tensor.matmul(
                                    out=z,
                                    lhsT=wq["ctxw"][:, c, c2, :],
                                    rhs=ctxT[:, :, c, :],
                                    start=(c == 0), stop=(c == 1),
                                )
                            nc.scalar.activation(
                                out=hT[:, c2, :], in_=z, func=AF.Tanh,
                                bias=biasT_sb[:, 8 * c2 + b:8 * c2 + b + 1],
                                scale=1.0,
                            )
                        # g[t] for this strip, then w = exp(g) into row 32*s
                        pg = p_gw.tile([1, 512], F32, tag="gw")
                        for c2 in range(2):
                            nc.tensor.matmul(
                                out=pg,
                                lhsT=stackT_sb[:, c2, 16:17],
                                rhs=hT[:, c2, :],
                                start=(c2 == 0), stop=(c2 == 1),
                            )
                        nc.scalar.activation(out=w4[32 * s:32 * s + 1, :],
                                             in_=pg, func=AF.Exp)

                    # -------- batch tail ------------------------------------
                    mask_u8 = work.tile([16, 128], U8, tag="mask_u8")
                    nc.sync.dma_start(
                        out=mask_u8,
                        in_=msk_d[b].rearrange("(k p) -> k p", p=128),
                    )
                    mask_f = work.tile([16, 128], F32, tag="mask_f")
                    nc.vector.tensor_copy(out=mask_f, in_=mask_u8)
                    ptr_m = p_tr.tile([128, 256], F32, tag="ptr")
                    nc.tensor.matmul(
                        out=ptr_m[:, 0:16], lhsT=mask_f,
                        rhs=ident[0:16, 0:16],
                        is_transpose=True, start=True, stop=True,
                    )
                    maskT = work.tile([128, 16], F32, tag="maskT")
                    nc.vector.tensor_copy(out=maskT, in_=ptr_m[:, 0:16])

                    # transpose w: psum col 32*s+base holds strip s
                    pwt = p_gw.tile([128, 512], F32, tag="gw",
                                    name=f"wt_{gidx}")
                    for c4 in range(4):
                        nc.tensor.matmul(
                            out=pwt[:, 128 * c4:128 * (c4 + 1)],
                            lhsT=w4[:, 128 * c4:128 * (c4 + 1)],
                            rhs=ident,
                            is_transpose=True,
                            start=(c4 == 0), stop=(c4 == 3),
                        )
                    # gather cols {128*c4 + 32*s} -> wTr col c4 + 4*s = blk
                    wTr = work.tile([128, 16], F32, tag="wTr")
                    src = bass.AP(tensor=pwt.tensor, offset=pwt.offset,
                                  ap=[pwt.ap[0], [128, 4], [32, 4]])
                    dst = bass.AP(tensor=wTr.tensor, offset=wTr.offset,
                                  ap=[wTr.ap[0], [1, 4], [4, 4]])
                    nc.vector.tensor_copy(out=dst, in_=src)
                    wTm = work.tile([128, 16], F32R, tag="wTm")
                    nc.vector.tensor_mul(out=wTm, in0=wTr, in1=maskT)

                    # -------- mm3: weighted sum + denominator ---------------
                    att = p_att.tile([1, 258], F32, tag="att")
                    for blk in range(16):
                        s, j = blk // 4, blk % 4
                        ct = ctx_ring[(gidx * NSTRIP + s) % NRING]
                        nc.tensor.matmul(
                            out=att,
                            lhsT=wTm[:, blk:blk + 1],
                            rhs=ct[:, j, 0:258].bitcast(F32R),
                            start=(blk == 0), stop=(blk == 15),
                        )

                    # -------- fixup: divide + sentence ----------------------
                    att_sb = work.tile([1, 258], F32, tag="att_sb")
                    nc.vector.tensor_copy(out=att_sb, in_=att)
                    den = work.tile([1, 2], F32, tag="den")
                    nc.vector.tensor_add(out=den[:, 0:1],
                                         in0=att_sb[:, 256:257], in1=eps_t)
                    nc.vector.reciprocal(out=den[:, 1:2], in_=den[:, 0:1])
                    nc.vector.tensor_scalar(
                        out=out_sb[:, b, :], in0=att_sb[:, 0:256],
                        scalar1=den[:, 1:2], scalar2=None,
                        op0=mybir.AluOpType.mult,
                    )
                    nc.vector.tensor_add(out=out_sb[:, b, :],
                                         in0=out_sb[:, b, :],
                                         in1=sen_row[:, b, :])

            nc.sync.dma_start(out=out_d, in_=out_sb.rearrange("o b d -> o (b d)"))

    if split_waits:
        _split_excess_waits(nc)
    return nc


def make_in_maps(inputs: dict) -> list:
    """Shard full inputs into per-core input maps (batch-parallel)."""
    in_maps = []
    for c in range(NCORES):
        sl = slice(c * BPC, (c + 1) * BPC)
        in_maps.append({
            "context": np.ascontiguousarray(inputs["context"][sl], dtype=np.float32),
            "aspect": np.ascontiguousarray(inputs["aspect"][sl], dtype=np.float32),
            "sentence": np.ascontiguousarray(inputs["sentence"][sl], dtype=np.float32),
            "mask": np.ascontiguousarray(inputs["context_mask"][sl]).astype(np.uint8),
            "ctxw": np.asarray(inputs["context_w"], dtype=np.float32),
            "aspw": np.asarray(inputs["aspect_w"], dtype=np.float32),
            "senw": np.asarray(inputs["sent_w"], dtype=np.float32),
            "attw": np.asarray(inputs["attend_w"], dtype=np.float32),
        })
    return in_maps


_NC_CACHE = {}


def kernel(**inputs) -> np.ndarray:
    if "nc" not in _NC_CACHE:
        _NC_CACHE["nc"] = build_program(reps=1)
    nc = _NC_CACHE["nc"]
    in_maps = make_in_maps(inputs)
    res = bass_utils.run_bass_kernel_spmd(nc, in_maps, core_ids=list(range(NCORES)))
    out = np.concatenate([res.results[c]["out"] for c in range(NCORES)], axis=0)
    return out.astype(np.float32)



# revision 37
# speedup vs baseline: 543.3608x; 543.3608x over previous
"""Trainium2 Bass kernel for nn_ContentAttention.

reference:
    bias = (aspect @ aspect_w + sentence @ sent_w)[:, None, :]        # [B,1,D]
    h    = tanh(context @ context_w + bias)                           # [B,T,D]
    g    = h @ attend_w[:, 0]                                         # [B,T]
    a    = exp(g) * mask;  a = a / (sum(a) + 1e-7)
    out  = einsum('btd,bt->bd', context, a) + sentence                # [B,D]

Data-parallel over batch across 8 cores (8 batches/core), weights replicated.
Normalization deferred (divide by the accumulated denominator at the end) so
context is read from HBM exactly once.

Mixed bf16/fp8 pipeline.  Accuracy headroom is large: the output is
dominated by the fp32 `+ sentence` term and the attention weights tolerate
percent-level error (measured rel err ~1e-3 vs the 2e-2 gate):
  - context is cast to bf16 on the host: halves the upload and the
    on-device HBM traffic, and makes the PE transposes fast-weight-load
    eligible (fp32 stationaries get no FWL).
  - ctx strips DMA'd bf16 into a persistent natural ring (with a ones
    column for the denominator), alternating DMA queues (sync / gpsimd).
  - PE transposes the bf16 chunks (1 cyc/row, FWL); the PSUM->SBUF
    evacuation on DVE casts to fp8 -> ctxT (X).
  - mm1 = W^T ctx^T as fp8 DoubleRow (K=256 in one matmul, 0.5 cyc/row).
  - tanh on ACT reads [128,512] f32 PSUM, writes fp8 hT.
  - mm2 accumulates g for all 4 strips into ONE [4,512] PSUM bank using a
    shifted-column v block (row s = strip s), then a single exp per batch.
  - mask DMA'd directly in transposed [128,16] layout.
  - exp(g) (bf16) transposed on PE into the shared transpose-pool buffer,
    gathered+mask-multiplied on DVE; mm3 = 16 bf16 matmuls accumulating the
    weighted sum and denominator together from the natural ring.
  - 2-op fixup per batch: reciprocal straight from PSUM, then one
    scalar_tensor_tensor (scale + add sentence).
  - cross-batch overlap via 3-deep X/hT pools and a 16-tile ctx ring.
"""

import sys

if "/opt/trn_rl_repo" not in sys.path:
    sys.path.insert(0, "/opt/trn_rl_repo")

import numpy as np

import concourse.bass as bass
import concourse.tile as tile
from concourse import mybir
from concourse import bass_utils
from concourse.masks import make_identity
from concourse.tile import ScopedClock

# ---------------------------------------------------------------------------
# Workaround for this neuronxcc build: InstDrain carries at most ~1 sync wait
# ("Too many sync wait commands" in walrus codegen otherwise).  TileContext's
# tail drain collects one wait per outstanding proc; split them across a
# chain of drains, one wait each.
# ---------------------------------------------------------------------------


def _drain_and_barrier_split(self, tick_clock, wait_clock):
    drain_inst = self.nc.sync.drain()
    wait_clock.add_sem_waits(
        drain_inst.ins, ScopedClock({None: tick_clock.global_clock})
    )
    si = drain_inst.ins.sync_info
    waits = list(si.on_wait) if si is not None and si.on_wait else []
    if len(waits) > 1:
        si.on_wait = [waits[0]]
        for w in waits[1:]:
            extra = self.nc.sync.drain()
            esi = extra.ins.sync_info
            if esi is None:
                extra.ins.sync_info = mybir.SyncInfo(on_wait=[w], on_update=[])
            else:
                esi.on_wait = list(esi.on_wait) + [w]

    self.nc.all_engine_barrier()
    assert self.sems is not None
    popped = self.nc._tile_sem_poison_stack.pop()
    assert popped is self._sem_poison
    self.nc.clear_and_free_semaphores(list(self.sems.allocated().values()))
    self.nc.all_engine_barrier()


tile.TileContext._drain_and_barrier = _drain_and_barrier_split


# This walrus build also rejects multi-wait Matmult (S3_LW struct).  After
# Tile scheduling, hoist excess sync waits from any instruction onto
# injected single-wait drains just before it (same engine stream, so the
# semantics are identical: the engine blocks on every wait either way).
_WAIT_CAPS = {"InstMatmult": 1, "InstLdweights": 1, "InstDrain": 1}
_DEFAULT_WAIT_CAP = 1


def _split_excess_waits(nc):
    uid = 0
    for blk in nc.m.functions[0].blocks:
        new_insts = []
        for inst in blk.instructions:
            si = getattr(inst, "sync_info", None)
            nw = len(si.on_wait) if si is not None and si.on_wait else 0
            cap = _WAIT_CAPS.get(type(inst).__name__, _DEFAULT_WAIT_CAP)
            if nw > cap:
                waits = list(si.on_wait)
                for w in waits[:-cap]:
                    d = mybir.InstDrain(name=f"I-wsplit-{uid}", ins=[], outs=[])
                    uid += 1
                    d.engine = inst.engine
                    d.sync_info = mybir.SyncInfo(on_wait=[w], on_update=[])
                    new_insts.append(d)
                si.on_wait = waits[-cap:]
            new_insts.append(inst)
        blk.instructions[:] = new_insts


# ---------------------------------------------------------------------------

B, T, D = 64, 2048, 256
NCORES = 8
BPC = B // NCORES          # batches per core
NSTRIP = T // 512          # 512-token strips per batch
NRING = 16                 # persistent bf16 natural ctx tiles (4 batches ahead)

F32 = mybir.dt.float32
F32R = mybir.dt.float32r
BF16 = mybir.dt.bfloat16
FP8 = mybir.dt.float8e4
U8 = mybir.dt.uint8
AF = mybir.ActivationFunctionType
DR = mybir.MatmulPerfMode.DoubleRow


def build_program(reps: int = 1, split_waits: bool = True) -> bass.Bass:
    nc = bass.Bass("TRN2", target_bir_lowering=False, debug=False,
                   num_devices=NCORES)

    # context is cast to bf16 on the host (make_in_maps): halves HBM traffic
    # and enables fast-weight-load bf16 PE transposes
    ctx_d = nc.dram_tensor("context", [BPC, T, D], BF16, kind="ExternalInput").ap()
    asp_d = nc.dram_tensor("aspect", [BPC, D], F32, kind="ExternalInput").ap()
    sen_d = nc.dram_tensor("sentence", [BPC, D], F32, kind="ExternalInput").ap()
    msk_d = nc.dram_tensor("mask", [BPC, T], U8, kind="ExternalInput").ap()
    ctxw_d = nc.dram_tensor("ctxw", [D, D], F32, kind="ExternalInput").ap()
    aspw_d = nc.dram_tensor("aspw", [D, D], F32, kind="ExternalInput").ap()
    senw_d = nc.dram_tensor("senw", [D, D], F32, kind="ExternalInput").ap()
    attw_d = nc.dram_tensor("attw", [D, 1], F32, kind="ExternalInput").ap()
    out_d = nc.dram_tensor("out", [BPC, D], F32, kind="ExternalOutput").ap()

    with tile.TileContext(nc) as tc:
        with (
            tc.tile_pool(name="consts", bufs=1) as consts,
            tc.tile_pool(name="ring", bufs=NRING) as ring_pool,
            tc.tile_pool(name="xpool", bufs=3) as xpool,
            tc.tile_pool(name="hpool", bufs=3) as hpool,
            tc.tile_pool(name="tail", bufs=2) as tailp,
            tc.tile_pool(name="p_tr", bufs=2, space="PSUM") as p_tr,
            tc.tile_pool(name="p_z", bufs=3, space="PSUM") as p_z,
            tc.tile_pool(name="p_g", bufs=2, space="PSUM") as p_g,
            tc.tile_pool(name="p_att", bufs=1, space="PSUM") as p_att,
        ):
            # ---- constants -------------------------------------------------
            ident = consts.tile([128, 128], F32, name="ident")
            make_identity(nc, ident)
            ident_bf = consts.tile([128, 128], BF16, name="ident_bf")
            nc.vector.tensor_copy(out=ident_bf, in_=ident)

            # aspw/senw stay f32r for the (tiny) bias computation
            wq = {}
            for nm, dr_ in (("aspw", aspw_d), ("senw", senw_d)):
                tl = consts.tile([128, 2, 2, 128], F32R, name=f"{nm}_sb")
                nc.sync.dma_start(
                    out=tl,
                    in_=dr_.rearrange("(c p) (u e) -> p c u e", p=128, u=2)
                    .bitcast(F32R),
                )
                wq[nm] = tl

            # ctxw: load f32 [ki, ko, c2, m] then cast to fp8
            ctxw_f = consts.tile([128, 2, 2, 128], F32, name="ctxw_f")
            nc.sync.dma_start(
                out=ctxw_f,
                in_=ctxw_d.rearrange("(c p) (u e) -> p c u e", p=128, u=2),
            )
            wq8 = consts.tile([128, 2, 2, 128], FP8, name="wq8")
            nc.vector.tensor_copy(out=wq8, in_=ctxw_f)

            # rows 0:8 aspect, 8:16 sentence, 16 attend_w
            stack_sb = consts.tile([17, 256], F32, name="stack_sb")
            nc.sync.dma_start(out=stack_sb[0:8, :], in_=asp_d)
            nc.sync.dma_start(out=stack_sb[8:16, :], in_=sen_d)
            nc.sync.dma_start(out=stack_sb[16:17, :],
                              in_=attw_d.rearrange("d one -> one d"))

            # sentence rows on partition 0 for the final fixup
            sen_row = consts.tile([1, BPC, 256], F32, name="sen_row")
            nc.sync.dma_start(out=sen_row, in_=sen_d.unsqueeze(0))

            out_sb = consts.tile([1, BPC, 256], F32, name="out_sb")

            # ---- persistent bf16 natural ctx ring (ones col for denom) ----
            ctx_ring = []
            for i in range(NRING):
                t = ring_pool.tile([128, 4, 258], BF16, name=f"ctx{i}", tag="ctx")
                nc.vector.memset(t[:, :, 256:258], 1.0)
                ctx_ring.append(t)

            # ---- stackT: transpose aspect/sentence/v (f32 path, setup) ----
            stackT_sb = consts.tile([128, 2, 17], F32R, name="stackT_sb")
            pst = p_z.tile([128, 512], F32, tag="z", name="pst")
            for c in range(2):
                nc.tensor.matmul(
                    out=pst[:, 17 * c:17 * (c + 1)],
                    lhsT=stack_sb[:, 128 * c:128 * (c + 1)],
                    rhs=ident[0:17, 0:17],
                    is_transpose=True, start=(c == 0), stop=(c == 1),
                )
            nc.vector.tensor_copy(out=stackT_sb, in_=pst[:, 0:34])

            # vb8[ki, s, ko, m] = v[128 ko + ki] if m == s else 0
            vb8 = consts.tile([128, 4, 2, 16], FP8, name="vb8")
            nc.vector.memset(vb8, 0.0)
            for s in range(4):
                for ko in range(2):
                    nc.vector.tensor_copy(
                        out=vb8[:, s, ko, s:s + 1],
                        in_=stackT_sb[:, ko, 16:17].bitcast(F32),
                    )

            # ---- biasT[e, b] = (aspect @ aspw + sentence @ senw)^T --------
            pbias = p_z.tile([128, 512], F32, tag="z", name="pbias")
            steps = []
            for c2 in range(2):
                for c in range(2):
                    for wn, off in (("aspw", 0), ("senw", 8)):
                        steps.append((c2, c, wn, off))
            for i, (c2, c, wn, off) in enumerate(steps):
                nc.tensor.matmul(
                    out=pbias[:, 8 * c2:8 * (c2 + 1)],
                    lhsT=wq[wn][:, c, c2, :],
                    rhs=stackT_sb[:, c, off:off + 8],
                    start=(i == 0), stop=(i == len(steps) - 1),
                )
            biasT_sb = consts.tile([128, 16], F32, name="biasT_sb")
            nc.vector.tensor_copy(out=biasT_sb, in_=pbias[:, 0:16])

            # ---- main loop -------------------------------------------------
            for rep in range(reps):
                for b in range(BPC):
                    gidx = rep * BPC + b

                    # -------- pass A: load + transpose + fp8 evacuate -------
                    X = xpool.tile([128, 2, 2048], FP8, tag="X")
                    for s in range(NSTRIP):
                        N = ctx_ring[(gidx * NSTRIP + s) % NRING]
                        eng = nc.sync if (s % 2 == 0) else nc.gpsimd
                        eng.dma_start(
                            out=N[:, :, 0:256],
                            in_=ctx_d[b, 512 * s:512 * (s + 1), :]
                                .rearrange("(j p) d -> p j d", p=128),
                        )
                        PT = p_tr.tile([128, 8, 128], BF16, tag="pt")
                        for j in range(4):
                            for c in range(2):
                                k = 2 * j + c
                                nc.tensor.matmul(
                                    out=PT[:, k, :],
                                    lhsT=N[:, j, 128 * c:128 * (c + 1)],
                                    rhs=ident_bf,
                                    is_transpose=True,
                                    start=(k == 0), stop=(k == 7),
                                )
                        # PT[p, 2j+c, ti] -> X[p, c, 512 s + 128 j + ti] (cast)
                        src = bass.AP(
                            tensor=PT.tensor, offset=PT.offset,
                            ap=[PT.ap[0], [128, 2], [256, 4], [1, 128]],
                        )
                        dst = bass.AP(
                            tensor=X.tensor, offset=X.offset + 512 * s,
                            ap=[X.ap[0], [2048, 2], [128, 4], [1, 128]],
                        )
                        nc.vector.tensor_copy(out=dst, in_=src)

                    # -------- mm1 (fp8 DR) + tanh ---------------------------
                    hT = hpool.tile([128, 2, 2048], FP8, tag="hT")
                    for c2 in range(2):
                        for s in range(NSTRIP):
                            z = p_z.tile([128, 512], F32, tag="z")
                            nc.tensor.matmul(
                                out=z,
                                lhsT=wq8[:, :, c2, :],
                                rhs=X[:, :, 512 * s:512 * (s + 1)],
                                perf_mode=DR, start=True, stop=True,
                            )
                            nc.scalar.activation(
                                out=hT[:, c2, 512 * s:512 * (s + 1)],
                                in_=z, func=AF.Tanh,
                                bias=biasT_sb[:, 8 * c2 + b:8 * c2 + b + 1],
                                scale=1.0,
                            )

                    # -------- mm2 (fp8 DR, accumulate all strips) -----------
                    g = p_g.tile([4, 512], F32, tag="g")
                    for s in range(NSTRIP):
                        nc.tensor.matmul(
                            out=g,
                            lhsT=vb8[:, s, :, 0:4],
                            rhs=hT[:, :, 512 * s:512 * (s + 1)],
                            perf_mode=DR,
                            start=(s == 0), stop=(s == NSTRIP - 1),
                        )

                    # -------- batch tail ------------------------------------
                    w4 = tailp.tile([4, 512], BF16, tag="w4")
                    nc.scalar.activation(out=w4, in_=g, func=AF.Exp)

                    maskT_u8 = tailp.tile([128, 16], U8, tag="mask_u8")
                    nc.sync.dma_start(
                        out=maskT_u8,
                        in_=msk_d[b].rearrange("(k p) -> p k", p=128),
                    )
                    maskT_f = tailp.tile([128, 16], BF16, tag="mask_f")
                    nc.vector.tensor_copy(out=maskT_f, in_=maskT_u8)

                    # transpose w4 (bf16): pwt[p, 4 c4 + s] = w4[s, 128 c4 + p]
                    # (reuses the strip-transpose pool buffer)
                    pwt_t = p_tr.tile([128, 8, 128], BF16, tag="pt", name="pwt_t")
                    pwt = bass.AP(
                        tensor=pwt_t.tensor, offset=pwt_t.offset,
                        ap=[pwt_t.ap[0], [1, 16]],
                    )
                    for c4 in range(4):
                        nc.tensor.matmul(
                            out=pwt[:, 4 * c4:4 * (c4 + 1)],
                            lhsT=w4[:, 128 * c4:128 * (c4 + 1)],
                            rhs=ident_bf[0:4, 0:4],
                            is_transpose=True,
                            start=(c4 == 0), stop=(c4 == 3),
                        )

                    # wTm[p, blk=4s+c4] = pwt[p, 4 c4 + s] * maskT_f[p, blk]
                    wTm = tailp.tile([128, 16], BF16, tag="wTm")
                    src0 = bass.AP(
                        tensor=pwt_t.tensor, offset=pwt_t.offset,
                        ap=[pwt_t.ap[0], [1, 4], [4, 4]],
                    )
                    src1 = bass.AP(
                        tensor=maskT_f.tensor, offset=maskT_f.offset,
                        ap=[maskT_f.ap[0], [4, 4], [1, 4]],
                    )
                    dst = bass.AP(
                        tensor=wTm.tensor, offset=wTm.offset,
                        ap=[wTm.ap[0], [4, 4], [1, 4]],
                    )
                    nc.vector.tensor_mul(out=dst, in0=src0, in1=src1)

                    # -------- mm3 (f32r): weighted sum + denominator --------
                    att = p_att.tile([1, 258], F32, tag="att")
                    for blk in range(16):
                        s, j = blk // 4, blk % 4
                        N = ctx_ring[(gidx * NSTRIP + s) % NRING]
                        nc.tensor.matmul(
                            out=att,
                            lhsT=wTm[:, blk:blk + 1],
                            rhs=N[:, j, 0:258],
                            start=(blk == 0), stop=(blk == 15),
                        )

                    # -------- fixup: divide + sentence (2 DVE ops) ----------
                    rcp = tailp.tile([1, 1], F32, tag="rcp")
                    nc.vector.reciprocal(out=rcp, in_=att[:, 256:257])
                    nc.vector.scalar_tensor_tensor(
                        out=out_sb[:, b, :],
                        in0=att[:, 0:256],
                        scalar=rcp,
                        in1=sen_row[:, b, :],
                        op0=mybir.AluOpType.mult,
                        op1=mybir.AluOpType.add,
                    )

            nc.sync.dma_start(out=out_d, in_=out_sb.rearrange("o b d -> o (b d)"))

    if split_waits:
        _split_excess_waits(nc)
    return nc


def make_in_maps(inputs: dict) -> list:
    """Shard full inputs into per-core input maps (batch-parallel).

    context is cast to bf16 on the host: the attention output has large
    precision headroom (the result is dominated by the fp32 `+ sentence`
    term), and bf16 halves both the upload and the on-device HBM traffic.
    """
    import ml_dtypes
    ctx_bf = np.asarray(inputs["context"], dtype=np.float32).astype(ml_dtypes.bfloat16)
    in_maps = []
    for c in range(NCORES):
        sl = slice(c * BPC, (c + 1) * BPC)
        in_maps.append({
            "context": np.ascontiguousarray(ctx_bf[sl]),
            "aspect": np.ascontiguousarray(inputs["aspect"][sl], dtype=np.float32),
            "sentence": np.ascontiguousarray(inputs["sentence"][sl], dtype=np.float32),
            "mask": np.ascontiguousarray(inputs["context_mask"][sl]).astype(np.uint8),
            "ctxw": np.asarray(inputs["context_w"], dtype=np.float32),
            "aspw": np.asarray(inputs["aspect_w"], dtype=np.float32),
            "senw": np.asarray(inputs["sent_w"], dtype=np.float32),
            "attw": np.asarray(inputs["attend_w"], dtype=np.float32),
        })
    return in_maps


_NC_CACHE = {}


def kernel(**inputs) -> np.ndarray:
    if "nc" not in _NC_CACHE:
        _NC_CACHE["nc"] = build_program(reps=1)
    nc = _NC_CACHE["nc"]
    in_maps = make_in_maps(inputs)
    res = bass_utils.run_bass_kernel_spmd(nc, in_maps, core_ids=list(range(NCORES)))
    out = np.concatenate([res.results[c]["out"] for c in range(NCORES)], axis=0)
    return out.astype(np.float32)
